# revision 25
# baseline (speedup 1.0000x reference)
"""YOLO-style loss kernel for Trainium2 (Bass/Tile), 8-core data-parallel, bf16.

Reference computation (per batch row, 7x7 grid, 30 pred ch / 25 target ch):
  p = predictions.reshape(B, 7, 7, 30); t = targets.reshape(B, 7, 7, 25)
  c1 = p[...,4]; c2 = p[...,9]; c = t[...,4]  (c is exactly 0.0/1.0)
  present = (c == 1.0);  r = c1 > c2
  obj  = sum(where(present, where(r,(c1-c)^2,(c2-c)^2), 0.5*(c1^2+c2^2)))
  cls  = sum(present * sum((p[...,10:30]-t[...,5:25])^2, -1))
  box  = 5*sum(present * (sum((pc-tc)^2,-1) + sum((sqrt(ph)-sqrt(th))^2,-1)))
  loss = obj + cls + box

Precision: inputs are converted to bf16 on the host (DMA traffic halves to
11.04 MB/core => ~30.7 us floor at the 360 B/ns DMA model); the loss gate is
rel_err < 2e-2 and bf16 keeps it ~1e-4.  1.0 is exact in bf16 so the
present mask (c == 1) survives quantization exactly.

Objectness algebra (masks exact; c IS the 0/1 present mask): with
e_i = (c_i - c)^2, obj = 0.5*sum(e1+e2) + sum(u*(e1-e2)), u = c*(r-0.5).
The e1+e2 sum falls out of the ACT Square's accum_out (host scales x0.5).

Engine split per 128-cell group (measured TimelineSim costs: DVE
TensorTensor bf16 0.53 ns/elem, InstCopy bf16 0.24, STT/reduce/copy_pred
1.06 dtype-blind, ACT 0.93, Pool ~2):
  Pool: 10 of 20 class-diff channels + cf diffs          (~26 ns/group)
  DVE : 10 class diffs, box select, tree-reduces, STT     (~32 ns/group)
  ACT : all squares (with free accum_out) + sqrts         (~28 ns/group)
All sit below the 39 ns/group DMA budget, so the stream stays DMA-bound.

Scheduling: engines resolve waits in program order, so iteration k emits
[DMAs(k), cls-tail(k-2), squares-tail(k-1), fresh compute(k)] - ready work
ahead of DMA-blocked work.  Tiles taper (128 rows first so compute starts
early, 256-row body, 128-row tail).  Host sums 8 x [128, NSLOT] partials
with per-slot scales in fp64.
"""

import math
from contextlib import ExitStack

import numpy as np

import concourse.bass as bass
import concourse.tile as tile
from concourse import mybir

B = 16384
N_CORES = 8
ROWS_PER_CORE = B // N_CORES  # 2048
P = 128  # partitions
PC = 1470  # prediction row length (49*30)
TC = 1225  # target row length (49*25)

# (row0, ql, cell_lo, cell_hi): per-core megatile schedule; a tile covers
# rows [row0, row0+ql*128) x grid cells [cell_lo, cell_hi).  Small tiles at
# the ends (fast ramp, short tail), big in the middle (amortize
# per-instruction engine overheads: ~57 ns/instr DVE, ~185 ns/instr ACT).
TILES = [(0, 1, 0, 49), (128, 1, 0, 49), (256, 2, 0, 49), (512, 2, 0, 49),
         (768, 2, 0, 49), (1024, 2, 0, 49), (1280, 2, 0, 49),
         (1536, 2, 0, 49), (1792, 1, 0, 49), (1920, 1, 0, 33), (1920, 1, 33, 49)]
assert sum(ql * P * (hi - lo) for _, ql, lo, hi in TILES) == ROWS_PER_CORE * 49

NSLOT = 2 * len(TILES)
SLOT_SCALES: list = []

F32 = mybir.dt.float32
BF16 = mybir.dt.bfloat16
U8 = mybir.dt.uint8
SQRT5 = math.sqrt(5.0)


def build_bass() -> bass.Bass:
    from concourse import bacc

    nc = bacc.Bacc("TRN2", target_bir_lowering=False)
    p_in = nc.dram_tensor("predictions", [ROWS_PER_CORE, PC], BF16, kind="ExternalInput")
    t_in = nc.dram_tensor("targets", [ROWS_PER_CORE, TC], BF16, kind="ExternalInput")
    out = nc.dram_tensor("partials", [P, NSLOT], F32, kind="ExternalOutput")

    with tile.TileContext(nc) as tc, ExitStack() as ctx:
        _yolo_loss_tile(ctx, tc, p_in, t_in, out)
    nc.compile()
    return nc


def _yolo_loss_tile(ctx, tc, p_in, t_in, out):
    nc = tc.nc
    io_p = ctx.enter_context(tc.tile_pool(name="io_p", bufs=5))
    io_t = ctx.enter_context(tc.tile_pool(name="io_t", bufs=5))
    work = ctx.enter_context(tc.tile_pool(name="work", bufs=5))
    singles = ctx.enter_context(tc.tile_pool(name="singles", bufs=1))

    accb = singles.tile([P, NSLOT], F32)
    nc.vector.memset(accb, 0.0)

    ADD, MUL = mybir.AluOpType.add, mybir.AluOpType.mult
    SQUARE = mybir.ActivationFunctionType.Square
    SQRT = mybir.ActivationFunctionType.Sqrt
    slot = [0]
    SLOT_SCALES.clear()

    def next_slot(scale=1.0):
        k = slot[0]
        slot[0] += 1
        SLOT_SCALES.append(scale)
        return accb[:, k : k + 1]

    p_ap = p_in[:, :]
    t_ap = t_in[:, :]

    def stage_dma(row0, ql, c_lo, c_hi):
        rows = ql * P
        cw = c_hi - c_lo
        nq = ql * cw
        p_t = io_p.tile([P, ql, cw * 30], BF16, tag="p_t")
        t_t = io_t.tile([P, ql, cw * 25], BF16, tag="t_t")
        nc.sync.dma_start(
            out=p_t,
            in_=p_ap[row0 : row0 + rows, c_lo * 30 : c_hi * 30].rearrange(
                "(q p) c -> p q c", p=P
            ),
        )
        nc.sync.dma_start(
            out=t_t,
            in_=t_ap[row0 : row0 + rows, c_lo * 25 : c_hi * 25].rearrange(
                "(q p) c -> p q c", p=P
            ),
        )
        return dict(nq=nq, p_t=p_t, t_t=t_t)

    def stage_a(s):
        """fresh-tile compute: diffs, select, sqrt, squares (ACT last)."""
        nq = s["nq"]
        pv = s["p_t"].rearrange("p q (c ch) -> p (q c) ch", ch=30)  # [P,nq,30]
        pg = s["p_t"].rearrange("p q (c g ch) -> p (q c) g ch", g=6, ch=5)
        tv = s["t_t"].rearrange("p q (c ch) -> p (q c) ch", ch=25)

        # d layout per cell: [0:20] cls diffs->squares, [16:21] gets the
        # tree-5 sums, [21] box cell sum, [22] obj (r-1/2)(e1-e2); the final
        # per-cell masked accum reduces [16:23] in one shot.
        d = work.tile([P, nq, 23], BF16, tag="d")
        A = work.tile([P, nq, 4], BF16, tag="A")  # selected box
        cf = work.tile([P, nq, 2], BF16, tag="cf")  # conf diffs -> e1,e2
        r8 = work.tile([P, nq], mybir.dt.uint16, tag="r8")
        gg = work.tile([P, nq], BF16, tag="gg")
        sqt = work.tile([P, nq, 2], BF16, tag="sqt")
        ccp = work.tile([P, nq], BF16, tag="ccp")  # copy of the present mask
        sc = work.tile([P, nq], F32, tag="sc")  # f32: tensor_reduce add demands it

        # pred-only ops first (pred DMA lands before targets)
        nc.vector.tensor_tensor(r8, pv[:, :, 4], pv[:, :, 9], op=mybir.AluOpType.is_gt)
        nc.vector.tensor_copy(A, pg[:, :, 1, 0:4])
        nc.vector.copy_predicated(
            A, r8.unsqueeze(2).broadcast_to([P, nq, 4]), pg[:, :, 0, 0:4]
        )
        nc.scalar.activation(A[:, :, 2:4], A[:, :, 2:4], SQRT)

        # target-dependent work; ccp copy first so io_t frees after stage_a
        # (stage_b/c only read work tiles).  Pool: mask copy, 12 cls ch, cf.
        nc.gpsimd.tensor_copy(ccp, tv[:, :, 4])
        nc.gpsimd.tensor_sub(d[:, :, 0:12], pv[:, :, 10:22], tv[:, :, 5:17])
        nc.vector.tensor_sub(d[:, :, 12:20], pv[:, :, 22:30], tv[:, :, 17:25])
        nc.gpsimd.tensor_sub(
            cf, pg[:, :, 0:2, 4], ccp.unsqueeze(2).broadcast_to([P, nq, 2])
        )
        nc.scalar.activation(sqt, tv[:, :, 2:4], SQRT)
        nc.gpsimd.tensor_sub(A[:, :, 0:2], A[:, :, 0:2], tv[:, :, 0:2])
        nc.vector.tensor_sub(A[:, :, 2:4], A[:, :, 2:4], sqt)

        # squares on ACT; conf accum gives sum(e1+e2) (host x0.5)
        nc.scalar.activation(cf, cf, SQUARE, accum_out=next_slot(0.5))
        nc.scalar.activation(A, A, SQUARE, scale=SQRT5)
        nc.scalar.activation(d[:, :, 0:20], d[:, :, 0:20], SQUARE)

        s.update(pv=pv, c=ccp, d=d, A=A, cf=cf, r8=r8, gg=gg, sc=sc)
        return s

    def stage_b(s):
        """obj + box cell terms (needs stage_a squares of this tile)."""
        nq = s["nq"]
        A, cf, d = s["A"], s["cf"], s["d"]
        # obj: (r-1/2)*(e1-e2) -> d[22]; c mask rides the shared accum
        nc.vector.tensor_sub(s["gg"], cf[:, :, 0], cf[:, :, 1])
        nc.vector.scalar_tensor_tensor(
            out=d[:, :, 22], in0=s["r8"], scalar=-0.5, in1=s["gg"],
            op0=ADD, op1=MUL,
        )
        # box per-cell sum (pair tree) -> d[21]
        nc.vector.tensor_add(A[:, :, 0:2], A[:, :, 0:2], A[:, :, 2:4])
        nc.vector.tensor_add(d[:, :, 21], A[:, :, 0], A[:, :, 1])

    def stage_c(s):
        """cls tail: tree-reduce 20 squares into [16:21], then fold box/obj
        ([21],[22]) in with paired adds and one masked accum: the per-cell
        total is cls + box + obj-delta, all c-masked."""
        nq, c, d = s["nq"], s["c"], s["d"]
        nc.vector.tensor_add(d[:, :, 0:10], d[:, :, 0:10], d[:, :, 10:20])
        nc.vector.tensor_add(d[:, :, 16:21], d[:, :, 0:5], d[:, :, 5:10])
        nc.vector.tensor_reduce(
            s["sc"], d[:, :, 16:23], axis=mybir.AxisListType.X, op=ADD
        )
        nc.vector.scalar_tensor_tensor(
            out=s["sc"], in0=s["sc"], scalar=1.0, in1=c,
            op0=MUL, op1=MUL, accum_out=next_slot(),
        )

    # Software pipeline, ready-work-first.
    hist = []
    for args in TILES:
        dmad = stage_dma(*args)
        if len(hist) >= 2:
            stage_c(hist[-2])
        if len(hist) >= 1:
            stage_b(hist[-1])
        hist.append(stage_a(dmad))
    stage_c(hist[-2])
    stage_b(hist[-1])
    stage_c(hist[-1])

    assert slot[0] == NSLOT, slot[0]
    nc.sync.dma_start(out=out[:, :], in_=accb)


_NC_CACHE = None


def _get_nc():
    global _NC_CACHE
    if _NC_CACHE is None:
        _NC_CACHE = build_bass()
    return _NC_CACHE


def run_sharded(predictions: np.ndarray, targets: np.ndarray, trace: bool = False):
    """Run the 8-core SPMD kernel; returns (total_loss, BassKernelResults)."""
    import ml_dtypes
    from concourse import bass_utils

    assert predictions.shape == (B, PC), predictions.shape
    assert targets.shape == (B, TC), targets.shape
    p16 = np.asarray(predictions, dtype=np.float32).astype(ml_dtypes.bfloat16)
    t16 = np.asarray(targets, dtype=np.float32).astype(ml_dtypes.bfloat16)

    nc = _get_nc()
    in_maps = []
    for i in range(N_CORES):
        sl = slice(i * ROWS_PER_CORE, (i + 1) * ROWS_PER_CORE)
        in_maps.append(
            {
                "predictions": np.ascontiguousarray(p16[sl]),
                "targets": np.ascontiguousarray(t16[sl]),
            }
        )
    res = bass_utils.run_bass_kernel_spmd(
        nc, in_maps, core_ids=list(range(N_CORES)), trace=trace
    )
    scales = np.asarray(SLOT_SCALES, np.float64)
    assert scales.shape == (NSLOT,)
    total = 0.0
    for r in res.results:
        partials = r["partials"].astype(np.float64)
        total += float(partials.sum(axis=0) @ scales)
    return np.float32(total), res


def kernel(predictions: np.ndarray, targets: np.ndarray) -> np.ndarray:
    total, _ = run_sharded(predictions, targets, trace=False)
    return np.array(total, dtype=np.float32)


# revision 28
# speedup vs baseline: 1.0162x; 1.0162x over previous
"""YOLO-style loss kernel for Trainium2 (Bass/Tile), 8-core data-parallel, bf16.

Reference computation (per batch row, 7x7 grid, 30 pred ch / 25 target ch):
  p = predictions.reshape(B, 7, 7, 30); t = targets.reshape(B, 7, 7, 25)
  c1 = p[...,4]; c2 = p[...,9]; c = t[...,4]  (c is exactly 0.0/1.0)
  present = (c == 1.0);  r = c1 > c2
  obj  = sum(where(present, where(r,(c1-c)^2,(c2-c)^2), 0.5*(c1^2+c2^2)))
  cls  = sum(present * sum((p[...,10:30]-t[...,5:25])^2, -1))
  box  = 5*sum(present * (sum((pc-tc)^2,-1) + sum((sqrt(ph)-sqrt(th))^2,-1)))
  loss = obj + cls + box

Precision: inputs are converted to bf16 on the host (DMA traffic halves to
11.04 MB/core => ~30.7 us floor at the 360 B/ns DMA model); the loss gate is
rel_err < 2e-2 and bf16 keeps it ~1e-4.  1.0 is exact in bf16 so the
present mask (c == 1) survives quantization exactly.

Objectness algebra (masks exact; c IS the 0/1 present mask): with
e_i = (c_i - c)^2, obj = 0.5*sum(e1+e2) + sum(u*(e1-e2)), u = c*(r-0.5).
The e1+e2 sum falls out of the ACT Square's accum_out (host scales x0.5).

Engine split per 128-cell group (measured TimelineSim costs: DVE
TensorTensor bf16 0.53 ns/elem, InstCopy bf16 0.24, STT/reduce/copy_pred
1.06 dtype-blind, ACT 0.93, Pool ~2):
  Pool: 10 of 20 class-diff channels + cf diffs          (~26 ns/group)
  DVE : 10 class diffs, box select, tree-reduces, STT     (~32 ns/group)
  ACT : all squares (with free accum_out) + sqrts         (~28 ns/group)
All sit below the 39 ns/group DMA budget, so the stream stays DMA-bound.

Scheduling: engines resolve waits in program order, so iteration k emits
[DMAs(k), cls-tail(k-2), squares-tail(k-1), fresh compute(k)] - ready work
ahead of DMA-blocked work.  Tiles taper (128 rows first so compute starts
early, 256-row body, 128-row tail).  Host sums 8 x [128, NSLOT] partials
with per-slot scales in fp64.
"""

import math
from contextlib import ExitStack

import numpy as np

import concourse.bass as bass
import concourse.tile as tile
from concourse import mybir

B = 16384
N_CORES = 8
ROWS_PER_CORE = B // N_CORES  # 2048
P = 128  # partitions
PC = 1470  # prediction row length (49*30)
TC = 1225  # target row length (49*25)

# (row0, ql, cell_lo, cell_hi): per-core megatile schedule; a tile covers
# rows [row0, row0+ql*128) x grid cells [cell_lo, cell_hi).  Small tiles at
# the ends (fast ramp, short tail), big in the middle (amortize
# per-instruction engine overheads: ~57 ns/instr DVE, ~185 ns/instr ACT).
TILES = [(0, 1, 0, 49), (128, 1, 0, 49), (256, 2, 0, 49), (512, 2, 0, 49),
         (768, 2, 0, 49), (1024, 2, 0, 49), (1280, 2, 0, 49),
         (1536, 2, 0, 49), (1792, 1, 0, 49), (1920, 1, 0, 49)]
assert sum(ql * P * (hi - lo) for _, ql, lo, hi in TILES) == ROWS_PER_CORE * 49

NSLOT = 2 * len(TILES)
SLOT_SCALES: list = []

F32 = mybir.dt.float32
BF16 = mybir.dt.bfloat16
U8 = mybir.dt.uint8
SQRT5 = math.sqrt(5.0)


def build_bass() -> bass.Bass:
    from concourse import bacc

    nc = bacc.Bacc("TRN2", target_bir_lowering=False)
    p_in = nc.dram_tensor("predictions", [ROWS_PER_CORE, PC], BF16, kind="ExternalInput")
    t_in = nc.dram_tensor("targets", [ROWS_PER_CORE, TC], BF16, kind="ExternalInput")
    out = nc.dram_tensor("partials", [P, NSLOT], F32, kind="ExternalOutput")

    with tile.TileContext(nc) as tc, ExitStack() as ctx:
        _yolo_loss_tile(ctx, tc, p_in, t_in, out)
    nc.compile()
    return nc


def _yolo_loss_tile(ctx, tc, p_in, t_in, out):
    nc = tc.nc
    io_p = ctx.enter_context(tc.tile_pool(name="io_p", bufs=5))
    io_t = ctx.enter_context(tc.tile_pool(name="io_t", bufs=5))
    work = ctx.enter_context(tc.tile_pool(name="work", bufs=5))
    singles = ctx.enter_context(tc.tile_pool(name="singles", bufs=1))

    accb = singles.tile([P, NSLOT], F32)
    nc.vector.memset(accb, 0.0)

    ADD, MUL = mybir.AluOpType.add, mybir.AluOpType.mult
    SQUARE = mybir.ActivationFunctionType.Square
    SQRT = mybir.ActivationFunctionType.Sqrt
    slot = [0]
    SLOT_SCALES.clear()

    def next_slot(scale=1.0):
        k = slot[0]
        slot[0] += 1
        SLOT_SCALES.append(scale)
        return accb[:, k : k + 1]

    p_ap = p_in[:, :]
    t_ap = t_in[:, :]

    def stage_dma(row0, ql, c_lo, c_hi):
        rows = ql * P
        cw = c_hi - c_lo
        nq = ql * cw
        p_t = io_p.tile([P, ql, cw * 30], BF16, tag="p_t")
        t_t = io_t.tile([P, ql, cw * 25], BF16, tag="t_t")
        nc.sync.dma_start(
            out=p_t,
            in_=p_ap[row0 : row0 + rows, c_lo * 30 : c_hi * 30].rearrange(
                "(q p) c -> p q c", p=P
            ),
        )
        nc.sync.dma_start(
            out=t_t,
            in_=t_ap[row0 : row0 + rows, c_lo * 25 : c_hi * 25].rearrange(
                "(q p) c -> p q c", p=P
            ),
        )
        return dict(nq=nq, p_t=p_t, t_t=t_t)

    def stage_a_pred(s):
        """pred-only compute (pred DMA lands before targets)."""
        nq = s["nq"]
        pv = s["p_t"].rearrange("p q (c ch) -> p (q c) ch", ch=30)  # [P,nq,30]
        pg = s["p_t"].rearrange("p q (c g ch) -> p (q c) g ch", g=6, ch=5)
        tv = s["t_t"].rearrange("p q (c ch) -> p (q c) ch", ch=25)

        # d layout per cell: [0:20] cls diffs->squares, [16:21] gets the
        # tree-5 sums, [21] box cell sum, [22] obj (r-1/2)(e1-e2); the final
        # per-cell masked accum reduces [16:23] in one shot.
        d = work.tile([P, nq, 23], BF16, tag="d")
        A = work.tile([P, nq, 4], BF16, tag="A")  # selected box
        cf = work.tile([P, nq, 2], BF16, tag="cf")  # conf diffs -> e1,e2
        r8 = work.tile([P, nq], mybir.dt.uint16, tag="r8")
        gg = work.tile([P, nq], BF16, tag="gg")
        sqt = work.tile([P, nq, 2], BF16, tag="sqt")
        ccp = work.tile([P, nq], BF16, tag="ccp")  # copy of the present mask
        sc = work.tile([P, nq], F32, tag="sc")  # f32: tensor_reduce add demands it

        nc.vector.tensor_tensor(r8, pv[:, :, 4], pv[:, :, 9], op=mybir.AluOpType.is_gt)
        nc.vector.tensor_copy(A, pg[:, :, 1, 0:4])
        nc.vector.copy_predicated(
            A, r8.unsqueeze(2).broadcast_to([P, nq, 4]), pg[:, :, 0, 0:4]
        )
        nc.scalar.activation(A[:, :, 2:4], A[:, :, 2:4], SQRT)

        s.update(pv=pv, pg=pg, tv=tv, c=ccp, d=d, A=A, cf=cf, r8=r8, gg=gg,
                 sqt=sqt, sc=sc)
        return s

    def stage_a_tgt(s):
        """target-dependent work; ccp copy first so io_t frees after this
        stage (stage_b/c only read work tiles).  Pool: mask copy, 12 cls
        channels, cf, box-center sub."""
        nq = s["nq"]
        pv, pg, tv = s["pv"], s["pg"], s["tv"]
        d, A, cf, ccp, sqt = s["d"], s["A"], s["cf"], s["c"], s["sqt"]

        nc.gpsimd.tensor_copy(ccp, tv[:, :, 4])
        nc.gpsimd.tensor_sub(d[:, :, 0:12], pv[:, :, 10:22], tv[:, :, 5:17])
        nc.vector.tensor_sub(d[:, :, 12:20], pv[:, :, 22:30], tv[:, :, 17:25])
        nc.gpsimd.tensor_sub(
            cf, pg[:, :, 0:2, 4], ccp.unsqueeze(2).broadcast_to([P, nq, 2])
        )
        nc.scalar.activation(sqt, tv[:, :, 2:4], SQRT)
        nc.gpsimd.tensor_sub(A[:, :, 0:2], A[:, :, 0:2], tv[:, :, 0:2])
        nc.vector.tensor_sub(A[:, :, 2:4], A[:, :, 2:4], sqt)

        # squares on ACT; conf accum gives sum(e1+e2) (host x0.5)
        nc.scalar.activation(cf, cf, SQUARE, accum_out=next_slot(0.5))
        nc.scalar.activation(A, A, SQUARE, scale=SQRT5)
        nc.scalar.activation(d[:, :, 0:20], d[:, :, 0:20], SQUARE)
        return s

    def stage_b(s):
        """obj + box cell terms (needs stage_a squares of this tile)."""
        nq = s["nq"]
        A, cf, d = s["A"], s["cf"], s["d"]
        # obj: (r-1/2)*(e1-e2) -> d[22]; c mask rides the shared accum
        nc.vector.tensor_sub(s["gg"], cf[:, :, 0], cf[:, :, 1])
        nc.vector.scalar_tensor_tensor(
            out=d[:, :, 22], in0=s["r8"], scalar=-0.5, in1=s["gg"],
            op0=ADD, op1=MUL,
        )
        # box per-cell sum (pair tree) -> d[21]
        nc.vector.tensor_add(A[:, :, 0:2], A[:, :, 0:2], A[:, :, 2:4])
        nc.vector.tensor_add(d[:, :, 21], A[:, :, 0], A[:, :, 1])

    def stage_c(s):
        """cls tail: tree-reduce 20 squares into [16:21], then fold box/obj
        ([21],[22]) in with paired adds and one masked accum: the per-cell
        total is cls + box + obj-delta, all c-masked."""
        nq, c, d = s["nq"], s["c"], s["d"]
        nc.vector.tensor_add(d[:, :, 0:10], d[:, :, 0:10], d[:, :, 10:20])
        nc.vector.tensor_add(d[:, :, 16:21], d[:, :, 0:5], d[:, :, 5:10])
        nc.vector.tensor_reduce(
            s["sc"], d[:, :, 16:23], axis=mybir.AxisListType.X, op=ADD
        )
        nc.vector.scalar_tensor_tensor(
            out=s["sc"], in0=s["sc"], scalar=1.0, in1=c,
            op0=MUL, op1=MUL, accum_out=next_slot(),
        )

    # Software pipeline, ready-work-first: iteration k emits
    # [DMAs(k), b(k-1), a_pred(k), c(k-1), a_tgt(k)] so the older tiles'
    # tails fill each engine's wait for tile k's transfers.
    hist = []
    for args in TILES:
        dmad = stage_dma(*args)
        if hist:
            stage_b(hist[-1])
        s = stage_a_pred(dmad)
        if hist:
            stage_c(hist[-1])
        hist.append(stage_a_tgt(s))
    stage_b(hist[-1])
    stage_c(hist[-1])

    assert slot[0] == NSLOT, slot[0]
    nc.sync.dma_start(out=out[:, :], in_=accb)


_NC_CACHE = None


def _get_nc():
    global _NC_CACHE
    if _NC_CACHE is None:
        _NC_CACHE = build_bass()
    return _NC_CACHE


def run_sharded(predictions: np.ndarray, targets: np.ndarray, trace: bool = False):
    """Run the 8-core SPMD kernel; returns (total_loss, BassKernelResults)."""
    import ml_dtypes
    from concourse import bass_utils

    assert predictions.shape == (B, PC), predictions.shape
    assert targets.shape == (B, TC), targets.shape
    p16 = np.asarray(predictions, dtype=np.float32).astype(ml_dtypes.bfloat16)
    t16 = np.asarray(targets, dtype=np.float32).astype(ml_dtypes.bfloat16)

    nc = _get_nc()
    in_maps = []
    for i in range(N_CORES):
        sl = slice(i * ROWS_PER_CORE, (i + 1) * ROWS_PER_CORE)
        in_maps.append(
            {
                "predictions": np.ascontiguousarray(p16[sl]),
                "targets": np.ascontiguousarray(t16[sl]),
            }
        )
    res = bass_utils.run_bass_kernel_spmd(
        nc, in_maps, core_ids=list(range(N_CORES)), trace=trace
    )
    scales = np.asarray(SLOT_SCALES, np.float64)
    assert scales.shape == (NSLOT,)
    total = 0.0
    for r in res.results:
        partials = r["partials"].astype(np.float64)
        total += float(partials.sum(axis=0) @ scales)
    return np.float32(total), res


def kernel(predictions: np.ndarray, targets: np.ndarray) -> np.ndarray:
    total, _ = run_sharded(predictions, targets, trace=False)
    return np.array(total, dtype=np.float32)


# revision 29
# speedup vs baseline: 1.0177x; 1.0014x over previous
"""YOLO-style loss kernel for Trainium2 (Bass/Tile), 8-core data-parallel, bf16.

Reference computation (per batch row, 7x7 grid, 30 pred ch / 25 target ch):
  p = predictions.reshape(B, 7, 7, 30); t = targets.reshape(B, 7, 7, 25)
  c1 = p[...,4]; c2 = p[...,9]; c = t[...,4]  (c is exactly 0.0/1.0)
  present = (c == 1.0);  r = c1 > c2
  obj  = sum(where(present, where(r,(c1-c)^2,(c2-c)^2), 0.5*(c1^2+c2^2)))
  cls  = sum(present * sum((p[...,10:30]-t[...,5:25])^2, -1))
  box  = 5*sum(present * (sum((pc-tc)^2,-1) + sum((sqrt(ph)-sqrt(th))^2,-1)))
  loss = obj + cls + box

Precision: inputs are converted to bf16 on the host (DMA traffic halves to
11.04 MB/core => ~30.7 us floor at the 360 B/ns DMA model); the loss gate is
rel_err < 2e-2 and bf16 keeps it ~1e-4.  1.0 is exact in bf16 so the
present mask (c == 1) survives quantization exactly.

Objectness algebra (masks exact; c IS the 0/1 present mask): with
e_i = (c_i - c)^2, obj = 0.5*sum(e1+e2) + sum(u*(e1-e2)), u = c*(r-0.5).
The e1+e2 sum falls out of the ACT Square's accum_out (host scales x0.5).

Engine split per 128-cell group (measured TimelineSim costs: DVE
TensorTensor bf16 0.53 ns/elem, InstCopy bf16 0.24, STT/reduce/copy_pred
1.06 dtype-blind, ACT 0.93, Pool ~2):
  Pool: 10 of 20 class-diff channels + cf diffs          (~26 ns/group)
  DVE : 10 class diffs, box select, tree-reduces, STT     (~32 ns/group)
  ACT : all squares (with free accum_out) + sqrts         (~28 ns/group)
All sit below the 39 ns/group DMA budget, so the stream stays DMA-bound.

Scheduling: engines resolve waits in program order, so iteration k emits
[DMAs(k), cls-tail(k-2), squares-tail(k-1), fresh compute(k)] - ready work
ahead of DMA-blocked work.  Tiles taper (128 rows first so compute starts
early, 256-row body, 128-row tail).  Host sums 8 x [128, NSLOT] partials
with per-slot scales in fp64.
"""

import math
from contextlib import ExitStack

import numpy as np

import concourse.bass as bass
import concourse.tile as tile
from concourse import mybir

B = 16384
N_CORES = 8
ROWS_PER_CORE = B // N_CORES  # 2048
P = 128  # partitions
PC = 1470  # prediction row length (49*30)
TC = 1225  # target row length (49*25)

# (row0, ql, cell_lo, cell_hi): per-core megatile schedule; a tile covers
# rows [row0, row0+ql*128) x grid cells [cell_lo, cell_hi).  Small tiles at
# the ends (fast ramp, short tail), big in the middle (amortize
# per-instruction engine overheads: ~57 ns/instr DVE, ~185 ns/instr ACT).
TILES = [(0, 1, 0, 49), (128, 1, 0, 49), (256, 2, 0, 49), (512, 2, 0, 49),
         (768, 2, 0, 49), (1024, 2, 0, 49), (1280, 2, 0, 49),
         (1536, 2, 0, 49), (1792, 1, 0, 49), (1920, 1, 0, 49)]
assert sum(ql * P * (hi - lo) for _, ql, lo, hi in TILES) == ROWS_PER_CORE * 49

NSLOT = 2 * len(TILES)
SLOT_SCALES: list = []

F32 = mybir.dt.float32
BF16 = mybir.dt.bfloat16
U8 = mybir.dt.uint8
SQRT5 = math.sqrt(5.0)


def build_bass() -> bass.Bass:
    from concourse import bacc

    nc = bacc.Bacc("TRN2", target_bir_lowering=False)
    p_in = nc.dram_tensor("predictions", [ROWS_PER_CORE, PC], BF16, kind="ExternalInput")
    t_in = nc.dram_tensor("targets", [ROWS_PER_CORE, TC], BF16, kind="ExternalInput")
    out = nc.dram_tensor("partials", [P, NSLOT], F32, kind="ExternalOutput")

    with tile.TileContext(nc) as tc, ExitStack() as ctx:
        _yolo_loss_tile(ctx, tc, p_in, t_in, out)
    nc.compile()
    return nc


def _yolo_loss_tile(ctx, tc, p_in, t_in, out):
    nc = tc.nc
    io_p = ctx.enter_context(tc.tile_pool(name="io_p", bufs=5))
    io_t = ctx.enter_context(tc.tile_pool(name="io_t", bufs=5))
    work = ctx.enter_context(tc.tile_pool(name="work", bufs=5))
    singles = ctx.enter_context(tc.tile_pool(name="singles", bufs=1))

    accb = singles.tile([P, NSLOT], F32)
    nc.vector.memset(accb, 0.0)

    ADD, MUL = mybir.AluOpType.add, mybir.AluOpType.mult
    SQUARE = mybir.ActivationFunctionType.Square
    SQRT = mybir.ActivationFunctionType.Sqrt
    slot = [0]
    SLOT_SCALES.clear()

    def next_slot(scale=1.0):
        k = slot[0]
        slot[0] += 1
        SLOT_SCALES.append(scale)
        return accb[:, k : k + 1]

    p_ap = p_in[:, :]
    t_ap = t_in[:, :]

    def stage_dma(row0, ql, c_lo, c_hi):
        rows = ql * P
        cw = c_hi - c_lo
        nq = ql * cw
        p_t = io_p.tile([P, ql, cw * 30], BF16, tag="p_t")
        t_t = io_t.tile([P, ql, cw * 25], BF16, tag="t_t")
        nc.sync.dma_start(
            out=p_t,
            in_=p_ap[row0 : row0 + rows, c_lo * 30 : c_hi * 30].rearrange(
                "(q p) c -> p q c", p=P
            ),
        )
        nc.sync.dma_start(
            out=t_t,
            in_=t_ap[row0 : row0 + rows, c_lo * 25 : c_hi * 25].rearrange(
                "(q p) c -> p q c", p=P
            ),
        )
        return dict(nq=nq, p_t=p_t, t_t=t_t)

    def stage_a_pred(s):
        """pred-only compute (pred DMA lands before targets)."""
        nq = s["nq"]
        pv = s["p_t"].rearrange("p q (c ch) -> p (q c) ch", ch=30)  # [P,nq,30]
        pg = s["p_t"].rearrange("p q (c g ch) -> p (q c) g ch", g=6, ch=5)
        tv = s["t_t"].rearrange("p q (c ch) -> p (q c) ch", ch=25)

        # d layout per cell: [0:20] cls diffs->squares, [16:21] gets the
        # tree-5 sums, [21] box cell sum, [22] obj (r-1/2)(e1-e2); the final
        # per-cell masked accum reduces [16:23] in one shot.
        d = work.tile([P, nq, 23], BF16, tag="d")
        A = work.tile([P, nq, 4], BF16, tag="A")  # selected box
        cf = work.tile([P, nq, 2], BF16, tag="cf")  # conf diffs -> e1,e2
        r8 = work.tile([P, nq], mybir.dt.uint16, tag="r8")
        gg = work.tile([P, nq], BF16, tag="gg")
        sqt = work.tile([P, nq, 2], BF16, tag="sqt")
        ccp = work.tile([P, nq], BF16, tag="ccp")  # copy of the present mask
        sc = work.tile([P, nq], F32, tag="sc")  # f32: tensor_reduce add demands it

        nc.vector.tensor_tensor(r8, pv[:, :, 4], pv[:, :, 9], op=mybir.AluOpType.is_gt)
        nc.vector.tensor_copy(A, pg[:, :, 1, 0:4])
        nc.vector.copy_predicated(
            A, r8.unsqueeze(2).broadcast_to([P, nq, 4]), pg[:, :, 0, 0:4]
        )
        nc.scalar.activation(A[:, :, 2:4], A[:, :, 2:4], SQRT)

        s.update(pv=pv, pg=pg, tv=tv, c=ccp, d=d, A=A, cf=cf, r8=r8, gg=gg,
                 sqt=sqt, sc=sc)
        return s

    def stage_a_tgt(s):
        """target-dependent work; ccp copy first so io_t frees after this
        stage (stage_b/c only read work tiles).  Pool: mask copy, 12 cls
        channels, cf, box-center sub."""
        nq = s["nq"]
        pv, pg, tv = s["pv"], s["pg"], s["tv"]
        d, A, cf, ccp, sqt = s["d"], s["A"], s["cf"], s["c"], s["sqt"]

        nc.gpsimd.tensor_copy(ccp, tv[:, :, 4])
        nc.gpsimd.tensor_sub(d[:, :, 0:13], pv[:, :, 10:23], tv[:, :, 5:18])
        nc.vector.tensor_sub(d[:, :, 13:20], pv[:, :, 23:30], tv[:, :, 18:25])
        nc.gpsimd.tensor_sub(
            cf, pg[:, :, 0:2, 4], ccp.unsqueeze(2).broadcast_to([P, nq, 2])
        )
        nc.scalar.activation(sqt, tv[:, :, 2:4], SQRT)
        nc.gpsimd.tensor_sub(A[:, :, 0:2], A[:, :, 0:2], tv[:, :, 0:2])
        nc.vector.tensor_sub(A[:, :, 2:4], A[:, :, 2:4], sqt)

        # squares on ACT; conf accum gives sum(e1+e2) (host x0.5)
        nc.scalar.activation(cf, cf, SQUARE, accum_out=next_slot(0.5))
        nc.scalar.activation(A, A, SQUARE, scale=SQRT5)
        nc.scalar.activation(d[:, :, 0:20], d[:, :, 0:20], SQUARE)
        return s

    def stage_b(s):
        """obj + box cell terms (needs stage_a squares of this tile)."""
        nq = s["nq"]
        A, cf, d = s["A"], s["cf"], s["d"]
        # obj: (r-1/2)*(e1-e2) -> d[22]; c mask rides the shared accum
        nc.vector.tensor_sub(s["gg"], cf[:, :, 0], cf[:, :, 1])
        nc.vector.scalar_tensor_tensor(
            out=d[:, :, 22], in0=s["r8"], scalar=-0.5, in1=s["gg"],
            op0=ADD, op1=MUL,
        )
        # box per-cell sum (pair tree) -> d[21]
        nc.vector.tensor_add(A[:, :, 0:2], A[:, :, 0:2], A[:, :, 2:4])
        nc.vector.tensor_add(d[:, :, 21], A[:, :, 0], A[:, :, 1])

    def stage_c(s):
        """cls tail: tree-reduce 20 squares into [16:21], then fold box/obj
        ([21],[22]) in with paired adds and one masked accum: the per-cell
        total is cls + box + obj-delta, all c-masked."""
        nq, c, d = s["nq"], s["c"], s["d"]
        nc.vector.tensor_add(d[:, :, 0:10], d[:, :, 0:10], d[:, :, 10:20])
        nc.vector.tensor_add(d[:, :, 16:21], d[:, :, 0:5], d[:, :, 5:10])
        nc.vector.tensor_reduce(
            s["sc"], d[:, :, 16:23], axis=mybir.AxisListType.X, op=ADD
        )
        nc.vector.scalar_tensor_tensor(
            out=s["sc"], in0=s["sc"], scalar=1.0, in1=c,
            op0=MUL, op1=MUL, accum_out=next_slot(),
        )

    # Software pipeline, ready-work-first: iteration k emits
    # [DMAs(k), b(k-1), a_pred(k), c(k-1), a_tgt(k)] so the older tiles'
    # tails fill each engine's wait for tile k's transfers.
    hist = []
    for args in TILES:
        dmad = stage_dma(*args)
        if hist:
            stage_b(hist[-1])
        s = stage_a_pred(dmad)
        if hist:
            stage_c(hist[-1])
        hist.append(stage_a_tgt(s))
    stage_b(hist[-1])
    stage_c(hist[-1])

    assert slot[0] == NSLOT, slot[0]
    nc.sync.dma_start(out=out[:, :], in_=accb)


_NC_CACHE = None


def _get_nc():
    global _NC_CACHE
    if _NC_CACHE is None:
        _NC_CACHE = build_bass()
    return _NC_CACHE


def run_sharded(predictions: np.ndarray, targets: np.ndarray, trace: bool = False):
    """Run the 8-core SPMD kernel; returns (total_loss, BassKernelResults)."""
    import ml_dtypes
    from concourse import bass_utils

    assert predictions.shape == (B, PC), predictions.shape
    assert targets.shape == (B, TC), targets.shape
    p16 = np.asarray(predictions, dtype=np.float32).astype(ml_dtypes.bfloat16)
    t16 = np.asarray(targets, dtype=np.float32).astype(ml_dtypes.bfloat16)

    nc = _get_nc()
    in_maps = []
    for i in range(N_CORES):
        sl = slice(i * ROWS_PER_CORE, (i + 1) * ROWS_PER_CORE)
        in_maps.append(
            {
                "predictions": np.ascontiguousarray(p16[sl]),
                "targets": np.ascontiguousarray(t16[sl]),
            }
        )
    res = bass_utils.run_bass_kernel_spmd(
        nc, in_maps, core_ids=list(range(N_CORES)), trace=trace
    )
    scales = np.asarray(SLOT_SCALES, np.float64)
    assert scales.shape == (NSLOT,)
    total = 0.0
    for r in res.results:
        partials = r["partials"].astype(np.float64)
        total += float(partials.sum(axis=0) @ scales)
    return np.float32(total), res


def kernel(predictions: np.ndarray, targets: np.ndarray) -> np.ndarray:
    total, _ = run_sharded(predictions, targets, trace=False)
    return np.array(total, dtype=np.float32)


# revision 30
# speedup vs baseline: 1.0201x; 1.0024x over previous
"""YOLO-style loss kernel for Trainium2 (Bass/Tile), 8-core data-parallel, bf16.

Reference computation (per batch row, 7x7 grid, 30 pred ch / 25 target ch):
  p = predictions.reshape(B, 7, 7, 30); t = targets.reshape(B, 7, 7, 25)
  c1 = p[...,4]; c2 = p[...,9]; c = t[...,4]  (c is exactly 0.0/1.0)
  present = (c == 1.0);  r = c1 > c2
  obj  = sum(where(present, where(r,(c1-c)^2,(c2-c)^2), 0.5*(c1^2+c2^2)))
  cls  = sum(present * sum((p[...,10:30]-t[...,5:25])^2, -1))
  box  = 5*sum(present * (sum((pc-tc)^2,-1) + sum((sqrt(ph)-sqrt(th))^2,-1)))
  loss = obj + cls + box

Precision: inputs are converted to bf16 on the host (DMA traffic halves to
11.04 MB/core => ~30.7 us floor at the 360 B/ns DMA model); the loss gate is
rel_err < 2e-2 and bf16 keeps it ~1e-4.  1.0 is exact in bf16 so the
present mask (c == 1) survives quantization exactly.

Objectness algebra (masks exact; c IS the 0/1 present mask): with
e_i = (c_i - c)^2, obj = 0.5*sum(e1+e2) + sum(u*(e1-e2)), u = c*(r-0.5).
The e1+e2 sum falls out of the ACT Square's accum_out (host scales x0.5).

Engine split per 128-cell group (measured TimelineSim costs: DVE
TensorTensor bf16 0.53 ns/elem, InstCopy bf16 0.24, STT/reduce/copy_pred
1.06 dtype-blind, ACT 0.93, Pool ~2):
  Pool: 10 of 20 class-diff channels + cf diffs          (~26 ns/group)
  DVE : 10 class diffs, box select, tree-reduces, STT     (~32 ns/group)
  ACT : all squares (with free accum_out) + sqrts         (~28 ns/group)
All sit below the 39 ns/group DMA budget, so the stream stays DMA-bound.

Scheduling: engines resolve waits in program order, so iteration k emits
[DMAs(k), cls-tail(k-2), squares-tail(k-1), fresh compute(k)] - ready work
ahead of DMA-blocked work.  Tiles taper (128 rows first so compute starts
early, 256-row body, 128-row tail).  Host sums 8 x [128, NSLOT] partials
with per-slot scales in fp64.
"""

import math
from contextlib import ExitStack

import numpy as np

import concourse.bass as bass
import concourse.tile as tile
from concourse import mybir

B = 16384
N_CORES = 8
ROWS_PER_CORE = B // N_CORES  # 2048
P = 128  # partitions
PC = 1470  # prediction row length (49*30)
TC = 1225  # target row length (49*25)

# (row0, ql, cell_lo, cell_hi): per-core megatile schedule; a tile covers
# rows [row0, row0+ql*128) x grid cells [cell_lo, cell_hi).  Small tiles at
# the ends (fast ramp, short tail), big in the middle (amortize
# per-instruction engine overheads: ~57 ns/instr DVE, ~185 ns/instr ACT).
TILES = [(0, 1, 0, 49), (128, 1, 0, 49), (256, 2, 0, 49), (512, 2, 0, 49),
         (768, 2, 0, 49), (1024, 2, 0, 49), (1280, 2, 0, 49),
         (1536, 2, 0, 49), (1792, 1, 0, 49), (1920, 1, 0, 49)]
assert sum(ql * P * (hi - lo) for _, ql, lo, hi in TILES) == ROWS_PER_CORE * 49

NSLOT = 2 * len(TILES)
SLOT_SCALES: list = []

F32 = mybir.dt.float32
BF16 = mybir.dt.bfloat16
U8 = mybir.dt.uint8
SQRT5 = math.sqrt(5.0)


def build_bass() -> bass.Bass:
    from concourse import bacc

    nc = bacc.Bacc("TRN2", target_bir_lowering=False)
    p_in = nc.dram_tensor("predictions", [ROWS_PER_CORE, PC], BF16, kind="ExternalInput")
    t_in = nc.dram_tensor("targets", [ROWS_PER_CORE, TC], BF16, kind="ExternalInput")
    out = nc.dram_tensor("partials", [P, NSLOT], F32, kind="ExternalOutput")

    with tile.TileContext(nc) as tc, ExitStack() as ctx:
        _yolo_loss_tile(ctx, tc, p_in, t_in, out)
    nc.compile()
    return nc


def _yolo_loss_tile(ctx, tc, p_in, t_in, out):
    nc = tc.nc
    io_p = ctx.enter_context(tc.tile_pool(name="io_p", bufs=5))
    io_t = ctx.enter_context(tc.tile_pool(name="io_t", bufs=5))
    work = ctx.enter_context(tc.tile_pool(name="work", bufs=5))
    singles = ctx.enter_context(tc.tile_pool(name="singles", bufs=1))

    accb = singles.tile([P, NSLOT], F32)
    nc.vector.memset(accb, 0.0)

    ADD, MUL = mybir.AluOpType.add, mybir.AluOpType.mult
    SQUARE = mybir.ActivationFunctionType.Square
    SQRT = mybir.ActivationFunctionType.Sqrt
    slot = [0]
    SLOT_SCALES.clear()

    def next_slot(scale=1.0):
        k = slot[0]
        slot[0] += 1
        SLOT_SCALES.append(scale)
        return accb[:, k : k + 1]

    p_ap = p_in[:, :]
    t_ap = t_in[:, :]

    def stage_dma(row0, ql, c_lo, c_hi):
        rows = ql * P
        cw = c_hi - c_lo
        nq = ql * cw
        p_t = io_p.tile([P, ql, cw * 30], BF16, tag="p_t")
        t_t = io_t.tile([P, ql, cw * 25], BF16, tag="t_t")
        nc.sync.dma_start(
            out=p_t,
            in_=p_ap[row0 : row0 + rows, c_lo * 30 : c_hi * 30].rearrange(
                "(q p) c -> p q c", p=P
            ),
        )
        nc.sync.dma_start(
            out=t_t,
            in_=t_ap[row0 : row0 + rows, c_lo * 25 : c_hi * 25].rearrange(
                "(q p) c -> p q c", p=P
            ),
        )
        return dict(nq=nq, p_t=p_t, t_t=t_t)

    def stage_a_pred(s):
        """pred-only compute (pred DMA lands before targets)."""
        nq = s["nq"]
        pv = s["p_t"].rearrange("p q (c ch) -> p (q c) ch", ch=30)  # [P,nq,30]
        pg = s["p_t"].rearrange("p q (c g ch) -> p (q c) g ch", g=6, ch=5)
        tv = s["t_t"].rearrange("p q (c ch) -> p (q c) ch", ch=25)

        # d layout per cell: [0:20] cls diffs->squares, [16:21] gets the
        # tree-5 sums, [21] box cell sum, [22] obj (r-1/2)(e1-e2); the final
        # per-cell masked accum reduces [16:23] in one shot.
        d = work.tile([P, nq, 23], BF16, tag="d")
        A = work.tile([P, nq, 4], BF16, tag="A")  # selected box
        cf = work.tile([P, nq, 2], BF16, tag="cf")  # conf diffs -> e1,e2
        r8 = work.tile([P, nq], mybir.dt.uint16, tag="r8")
        gg = work.tile([P, nq], BF16, tag="gg")
        sqt = work.tile([P, nq, 2], BF16, tag="sqt")
        ccp = work.tile([P, nq], BF16, tag="ccp")  # copy of the present mask
        sc = work.tile([P, nq], F32, tag="sc")  # f32: tensor_reduce add demands it

        nc.vector.tensor_tensor(r8, pv[:, :, 4], pv[:, :, 9], op=mybir.AluOpType.is_gt)
        nc.vector.tensor_copy(A, pg[:, :, 1, 0:4])
        nc.vector.copy_predicated(
            A, r8.unsqueeze(2).broadcast_to([P, nq, 4]), pg[:, :, 0, 0:4]
        )
        nc.scalar.activation(A[:, :, 2:4], A[:, :, 2:4], SQRT)

        s.update(pv=pv, pg=pg, tv=tv, c=ccp, d=d, A=A, cf=cf, r8=r8, gg=gg,
                 sqt=sqt, sc=sc)
        return s

    def stage_a_tgt(s):
        """target-dependent work; ccp copy first so io_t frees after this
        stage (stage_b/c only read work tiles).  Pool: mask copy, 12 cls
        channels, cf, box-center sub."""
        nq = s["nq"]
        pv, pg, tv = s["pv"], s["pg"], s["tv"]
        d, A, cf, ccp, sqt = s["d"], s["A"], s["cf"], s["c"], s["sqt"]

        nc.gpsimd.tensor_copy(ccp, tv[:, :, 4])
        nc.gpsimd.tensor_sub(d[:, :, 0:14], pv[:, :, 10:24], tv[:, :, 5:19])
        nc.vector.tensor_sub(d[:, :, 14:20], pv[:, :, 24:30], tv[:, :, 19:25])
        nc.gpsimd.tensor_sub(
            cf, pg[:, :, 0:2, 4], ccp.unsqueeze(2).broadcast_to([P, nq, 2])
        )
        nc.scalar.activation(sqt, tv[:, :, 2:4], SQRT)
        nc.gpsimd.tensor_sub(A[:, :, 0:2], A[:, :, 0:2], tv[:, :, 0:2])
        nc.vector.tensor_sub(A[:, :, 2:4], A[:, :, 2:4], sqt)

        # squares on ACT; conf accum gives sum(e1+e2) (host x0.5)
        nc.scalar.activation(cf, cf, SQUARE, accum_out=next_slot(0.5))
        nc.scalar.activation(A, A, SQUARE, scale=SQRT5)
        nc.scalar.activation(d[:, :, 0:20], d[:, :, 0:20], SQUARE)
        return s

    def stage_b(s):
        """obj + box cell terms (needs stage_a squares of this tile)."""
        nq = s["nq"]
        A, cf, d = s["A"], s["cf"], s["d"]
        # obj: (r-1/2)*(e1-e2) -> d[22]; c mask rides the shared accum
        nc.vector.tensor_sub(s["gg"], cf[:, :, 0], cf[:, :, 1])
        nc.vector.scalar_tensor_tensor(
            out=d[:, :, 22], in0=s["r8"], scalar=-0.5, in1=s["gg"],
            op0=ADD, op1=MUL,
        )
        # box per-cell sum (pair tree) -> d[21]
        nc.vector.tensor_add(A[:, :, 0:2], A[:, :, 0:2], A[:, :, 2:4])
        nc.vector.tensor_add(d[:, :, 21], A[:, :, 0], A[:, :, 1])

    def stage_c(s):
        """cls tail: tree-reduce 20 squares into [16:21], then fold box/obj
        ([21],[22]) in with paired adds and one masked accum: the per-cell
        total is cls + box + obj-delta, all c-masked."""
        nq, c, d = s["nq"], s["c"], s["d"]
        nc.vector.tensor_add(d[:, :, 0:10], d[:, :, 0:10], d[:, :, 10:20])
        nc.vector.tensor_add(d[:, :, 16:21], d[:, :, 0:5], d[:, :, 5:10])
        nc.vector.tensor_reduce(
            s["sc"], d[:, :, 16:23], axis=mybir.AxisListType.X, op=ADD
        )
        nc.vector.scalar_tensor_tensor(
            out=s["sc"], in0=s["sc"], scalar=1.0, in1=c,
            op0=MUL, op1=MUL, accum_out=next_slot(),
        )

    # Software pipeline, ready-work-first: iteration k emits
    # [DMAs(k), b(k-1), a_pred(k), c(k-1), a_tgt(k)] so the older tiles'
    # tails fill each engine's wait for tile k's transfers.
    hist = []
    for args in TILES:
        dmad = stage_dma(*args)
        if hist:
            stage_b(hist[-1])
        s = stage_a_pred(dmad)
        if hist:
            stage_c(hist[-1])
        hist.append(stage_a_tgt(s))
    stage_b(hist[-1])
    stage_c(hist[-1])

    assert slot[0] == NSLOT, slot[0]
    nc.sync.dma_start(out=out[:, :], in_=accb)


_NC_CACHE = None


def _get_nc():
    global _NC_CACHE
    if _NC_CACHE is None:
        _NC_CACHE = build_bass()
    return _NC_CACHE


def run_sharded(predictions: np.ndarray, targets: np.ndarray, trace: bool = False):
    """Run the 8-core SPMD kernel; returns (total_loss, BassKernelResults)."""
    import ml_dtypes
    from concourse import bass_utils

    assert predictions.shape == (B, PC), predictions.shape
    assert targets.shape == (B, TC), targets.shape
    p16 = np.asarray(predictions, dtype=np.float32).astype(ml_dtypes.bfloat16)
    t16 = np.asarray(targets, dtype=np.float32).astype(ml_dtypes.bfloat16)

    nc = _get_nc()
    in_maps = []
    for i in range(N_CORES):
        sl = slice(i * ROWS_PER_CORE, (i + 1) * ROWS_PER_CORE)
        in_maps.append(
            {
                "predictions": np.ascontiguousarray(p16[sl]),
                "targets": np.ascontiguousarray(t16[sl]),
            }
        )
    res = bass_utils.run_bass_kernel_spmd(
        nc, in_maps, core_ids=list(range(N_CORES)), trace=trace
    )
    scales = np.asarray(SLOT_SCALES, np.float64)
    assert scales.shape == (NSLOT,)
    total = 0.0
    for r in res.results:
        partials = r["partials"].astype(np.float64)
        total += float(partials.sum(axis=0) @ scales)
    return np.float32(total), res


def kernel(predictions: np.ndarray, targets: np.ndarray) -> np.ndarray:
    total, _ = run_sharded(predictions, targets, trace=False)
    return np.array(total, dtype=np.float32)
